# revision 20
# baseline (speedup 1.0000x reference)
"""Causal multi-head self-attention on 8 Trainium2 NeuronCores.

Problem: B=4, T=2048, C=1024, H=16 heads (d=64), fp32 in/out.
    q/k/v = x @ W{q,k,v}.T + b;  S = causal softmax(q k^T / sqrt(d));  y = (S v) @ Wo.T + bo

Sharding (8 cores): 2-D  (batch x head-group).
    core c -> batch b = c // 2, head-group g = c % 2 (8 heads / 512 features).
    Each core computes its batch's attention for its 8 heads plus the partial
    output projection against Wo[:, 512g:512g+512]; the host sums the two
    partials per batch and adds bo.

Device kernel (per core, identical SPMD program, Bass/Tile):
  - Projection/S operands are bf16 (fast-weight-load; the two K=64 S
    matmuls of a head pair run concurrently on PE row-tiles 0/64 at ~108ns
    each). E and V stay fp32r: the ACT exp (the attention-phase pacer)
    writes fp32 ~17% faster than bf16.
  - The attention inner loop is exp(ACT)-paced at ~940ns per 128-token
    j-step while the PE's matmul work per step is only ~650ns, so the
    remaining projection / output-projection matmuls are hand-interleaved
    into the emission stream in ~2-matmul units ("pumping") between every
    S-pair and PV group. This keeps the PE instruction queue free of
    head-of-line blocking on the exp chain and keeps both engines >90%
    busy simultaneously.
  - Warmup: dummy matmuls + a dummy exp during the initial DMA wait (PE
    HAM clock at 2.4 GHz, ACT exp table loaded before the first real op).

All host-side work is layout only (transpose/slice/cast) + the final
pairwise partial-sum.
"""

import math
import os

import numpy as np

os.environ.setdefault("JAX_COMPILATION_CACHE_DIR", "/tmp/jax_comp_cache")

B, T, C, H = 4, 2048, 1024, 16
D = C // H  # 64
NCORES = 8
GROUPS = 2  # head-groups (tensor parallel dimension)
HG = H // GROUPS  # heads per core = 8
CG = C // GROUPS  # features per core = 512
SCALE = 1.0 / math.sqrt(D)
P = 128
TCH = 512  # query chunk / matmul free dim
NTCH = T // TCH  # 4
NHP = CG // P  # 4 head-pairs per core

_MODULE_CACHE = {}


def _build_module(mm_fast=True):
    import concourse.bass as bass  # noqa: F401
    import concourse.mybir as mybir
    import concourse.tile as tile
    from concourse import bacc

    f32 = mybir.dt.float32
    bdt = mybir.dt.bfloat16  # q/k/proj-weight/x/ot dtype
    edt = mybir.dt.float32r  # E and V dtype (PV matmul operands)
    Exp = mybir.ActivationFunctionType.Exp

    nc = bacc.Bacc(None, target_bir_lowering=False)

    xt = nc.dram_tensor("xt", [C, T], bdt, kind="ExternalInput")
    wqt = nc.dram_tensor("wqt", [C, CG], bdt, kind="ExternalInput")
    wkt = nc.dram_tensor("wkt", [C, CG], bdt, kind="ExternalInput")
    wvt = nc.dram_tensor("wvt", [C, CG], bdt, kind="ExternalInput")
    wot = nc.dram_tensor("wot", [CG, C], bdt, kind="ExternalInput")
    bq2 = nc.dram_tensor("bq2", [P, NHP], f32, kind="ExternalInput")
    bk2 = nc.dram_tensor("bk2", [P, NHP], f32, kind="ExternalInput")
    bvb = nc.dram_tensor("bvb", [P, CG], f32, kind="ExternalInput")
    out = nc.dram_tensor("out", [T, C], bdt, kind="ExternalOutput")

    xt_r = xt.ap().rearrange("(cs p) t -> p cs t", p=P)  # [128, 8, 2048]
    wqt_r = wqt.ap().rearrange("(cs p) j -> p cs j", p=P)  # [128, 8, 512]
    wkt_r = wkt.ap().rearrange("(cs p) j -> p cs j", p=P)
    wvt_r = wvt.ap().rearrange("(cs p) j -> p cs j", p=P)
    wot_r = wot.ap().rearrange("(hp p) m -> p hp m", p=P)  # [128, 4, 1024]
    out_ap = out.ap()

    with tile.TileContext(nc) as tc:
        with (
            tc.tile_pool(name="persist", bufs=1) as persist,
            tc.tile_pool(name="smalls", bufs=1) as smalls,
            tc.tile_pool(name="qpool", bufs=2) as qpool,
            tc.tile_pool(name="xp", bufs=2) as xp,
            tc.tile_pool(name="otp", bufs=3) as otp,
            tc.tile_pool(name="ep", bufs=8) as ep,
            tc.tile_pool(name="npool", bufs=3) as npool,
            tc.tile_pool(name="psS", bufs=2, space="PSUM") as psS,
            tc.tile_pool(name="psO", bufs=2, space="PSUM") as psO,
            tc.tile_pool(name="ps3", bufs=2, space="PSUM") as ps3p,
        ):
            kT_t = []  # [feature-partition, head-pair, token] per chunk
            vx_t = []  # [token-partition, token-tile, head, d+1] per chunk
            for c in range(NTCH):
                kT_t.append(persist.tile([P, NHP, TCH], bdt, name=f"kT{c}"))
                vx_t.append(
                    persist.tile([P, TCH // P, HG, D + 1], edt, name=f"vx{c}")
                )

            # ---- warmup: PE HAM ramp + ACT exp-table preload while the
            # initial DMAs stream (~90 small matmuls = ~5us of PE busy).
            wu = smalls.tile([P, P], bdt, name="wu")
            nc.vector.memset(wu, 0.125)
            wups = ps3p.tile([P, 64], f32, tag="pso3", name="wups")
            for i in range(130):
                nc.tensor.matmul(
                    wups, wu, wu[:, 0:64], start=(i == 0), stop=(i == 129)
                )
            wue = smalls.tile([P, 1], f32, name="wue")
            nc.scalar.activation(wue, wups[:, 0:1], Exp, scale=SCALE)

            bqs = smalls.tile([P, NHP], f32)
            nc.sync.dma_start(bqs, bq2.ap())
            bks = smalls.tile([P, NHP], f32)
            nc.sync.dma_start(bks, bk2.ap())
            bvbs = smalls.tile([P, CG], f32)
            nc.sync.dma_start(bvbs, bvb.ap())

            # x chunk 0 leads the DMA queue split in cs-pairs (the first
            # q chain needs only cs 0..1); weights follow in first-use
            # order. wot is only needed at outproj(0).
            wqts = persist.tile([P, 8, CG], bdt, name="wqts")
            wkts = persist.tile([P, 8, CG], bdt, name="wkts")
            wvts = persist.tile([P, 8, CG], bdt, name="wvts")
            wots = persist.tile([P, NHP, C], bdt, name="wots")

            # 15 critical pieces (x chunk 0 + Wq) spread over the DMA queues
            # first; wk/wv/wot pieces land behind them in queue FIFO order
            # so the critical path gets the full HBM bandwidth.
            xtt0 = xp.tile([P, 8, TCH], bdt, tag="xtt", name="xtt0")
            for cs in range(8):
                nc.sync.dma_start(
                    xtt0[:, cs : cs + 1, :], xt_r[:, cs : cs + 1, 0:TCH]
                )
            for j4 in range(4):
                nc.sync.dma_start(
                    wqts[:, :, 128 * j4 : 128 * (j4 + 1)],
                    wqt_r[:, :, 128 * j4 : 128 * (j4 + 1)],
                )
            nc.sync.dma_start(wkts[:, :, 0:256], wkt_r[:, :, 0:256])
            nc.sync.dma_start(wkts[:, :, 256:512], wkt_r[:, :, 256:512])
            nc.sync.dma_start(wvts[:, 0:4, :], wvt_r[:, 0:4, :])
            nc.sync.dma_start(wvts[:, 4:8, :], wvt_r[:, 4:8, :])
            nc.sync.dma_start(wots[:, 0:2, :], wot_r[:, 0:2, :])
            nc.sync.dma_start(wots[:, 2:4, :], wot_r[:, 2:4, :])

            qT_cur = {}  # chunk -> qT tile (2-buf rotation)
            xtt_cur = {0: xtt0}
            ot_t = {}

            # ---- pump queue: generators emitting proj/outproj matmuls in
            # small units so they weave between exp-paced attention ops.
            pending = []

            def drain(tag):
                for g in [g for t, g in pending if t == tag]:
                    for _ in g:
                        pass
                pending[:] = [(t, g) for t, g in pending if t != tag]

            # pending entries are (tag, gen); pump operates FIFO
            def pump2(n):
                while n > 0 and pending:
                    tag, g = pending[0]
                    try:
                        next(g)
                        n -= 1
                    except StopIteration:
                        pending.pop(0)

            def proj_slice_gen(c, sl):
                """q/k feature-slice sl and v token-tile sl of chunk c;
                yields every 2 matmuls."""
                if sl == 0:
                    if c > 0:
                        xtt = xp.tile(
                            [P, 8, TCH], bdt, tag="xtt", name=f"xtt{c}"
                        )
                        nc.sync.dma_start(
                            xtt, xt_r[:, :, TCH * c : TCH * (c + 1)]
                        )
                        xtt_cur[c] = xtt
                    qT_cur[c] = qpool.tile(
                        [P, NHP, TCH], bdt, tag="qT", name=f"qT{c}"
                    )
                    # ones columns of v_ext (softmax-denominator trick);
                    # memset can't write float32r -> DVE 0*x + 1.
                    nc.vector.tensor_scalar(
                        vx_t[c][:, :, :, D],
                        bvbs[:, 0 : (TCH // P) * HG].rearrange(
                            "p (a b) -> p a b", b=HG
                        ),
                        0.0,
                        1.0,
                        mybir.AluOpType.mult,
                        mybir.AluOpType.add,
                    )
                xtt = xtt_cur[c]
                jsl = slice(P * sl, P * (sl + 1))
                psq = ps3p.tile([P, TCH], f32, tag="pso3", name=f"psq{c}_{sl}")
                for cs in range(8):
                    nc.tensor.matmul(
                        psq,
                        wqts[:, cs, jsl],
                        xtt[:, cs, :],
                        start=(cs == 0),
                        stop=(cs == 7),
                    )
                    if cs % 2 == 1:
                        yield
                nc.vector.tensor_scalar_add(
                    qT_cur[c][:, sl, :], psq, bqs[:, sl : sl + 1]
                )
                psk = ps3p.tile([P, TCH], f32, tag="pso3", name=f"psk{c}_{sl}")
                for cs in range(8):
                    nc.tensor.matmul(
                        psk,
                        wkts[:, cs, jsl],
                        xtt[:, cs, :],
                        start=(cs == 0),
                        stop=(cs == 7),
                    )
                    if cs % 2 == 1:
                        yield
                nc.vector.tensor_scalar_add(
                    kT_t[c][:, sl, :], psk, bks[:, sl : sl + 1]
                )
                psv = ps3p.tile([P, CG], f32, tag="pso3", name=f"psv{c}_{sl}")
                for cs in range(8):
                    nc.tensor.matmul(
                        psv,
                        xtt[:, cs, P * sl : P * (sl + 1)],
                        wvts[:, cs, :],
                        start=(cs == 0),
                        stop=(cs == 7),
                    )
                    if cs % 2 == 1:
                        yield
                nc.vector.tensor_add(
                    vx_t[c][:, sl, :, 0:D],
                    psv.rearrange("p (h d) -> p h d", d=D),
                    bvbs.rearrange("p (h d) -> p h d", d=D),
                )

            def outproj_gen(ic):
                otn = ot_t.pop(ic)
                for tt in range(TCH // P):
                    trow = TCH * ic + P * tt
                    for mi in range(C // TCH):
                        msl = slice(TCH * mi, TCH * (mi + 1))
                        pso3 = ps3p.tile(
                            [P, TCH], f32, tag="pso3", name=f"ps3{ic}_{tt}_{mi}"
                        )
                        for hp in range(NHP):
                            nc.tensor.matmul(
                                pso3,
                                otn[:, hp, P * tt : P * (tt + 1)],
                                wots[:, hp, msl],
                                start=(hp == 0),
                                stop=(hp == NHP - 1),
                            )
                            if hp % 2 == 1:
                                yield
                        osb = ep.tile(
                            [P, TCH], bdt, tag="osb", name=f"ob{ic}_{tt}_{mi}"
                        )
                        nc.vector.tensor_copy(osb, pso3)
                        nc.sync.dma_start(out_ap[trow : trow + P, msl], osb)

            def attention(ic, hp):
                if hp == 0:
                    ot_t[ic] = otp.tile(
                        [P, NHP, TCH], bdt, tag="ot", name=f"ot{ic}"
                    )
                njt = 4 * (ic + 1)
                qT = qT_cur[ic]

                # pass A: S^T pairs (64-row tiles T0/T8, concurrent) + exp.
                # High priority: the S->exp chain is the ACT engine's
                # feedstock; the scheduler must slot these ahead of any
                # ready proj/outproj matmuls or the ACT starves.
                ees = []
                for jt in range(njt):
                    cj, lj = jt // 4, jt % 4
                    r = jt - 4 * ic  # >= 0 only for diagonal tiles
                    lo = 0 if r <= 0 else P * r  # cols < 128r fully masked
                    psp = psS.tile(
                        [P, 2, TCH], f32, tag="psp", name=f"psp{ic}_{hp}_{jt}"
                    )
                    with tc.high_priority(offset=1_000_000):
                        for h01 in range(2):
                            pb = 64 * h01
                            nc.tensor.matmul(
                                psp[:, h01, lo:],
                                kT_t[cj][pb : pb + D, hp, P * lj : P * (lj + 1)],
                                qT[pb : pb + D, hp, lo:],
                                start=True,
                                stop=True,
                            )
                        ee = ep.tile(
                            [P, 2, TCH], edt, tag="ee", name=f"ee{ic}_{hp}_{jt}"
                        )
                        if r <= 0:
                            nc.scalar.activation(ee, psp, Exp, scale=SCALE)
                        else:
                            nc.scalar.activation(
                                ee[:, :, lo:], psp[:, :, lo:], Exp, scale=SCALE
                            )
                        if r >= 0:
                            # boundary 128 columns: keep where -p + f >= 0
                            bsl = slice(P * r, P * (r + 1))
                            nc.gpsimd.affine_select(
                                out=ee[:, :, bsl],
                                in_=ee[:, :, bsl],
                                compare_op=mybir.AluOpType.is_ge,
                                fill=0.0,
                                base=0,
                                pattern=[[0, 2], [1, P]],
                                channel_multiplier=-1,
                            )
                    ees.append(ee)
                    pump2(2)

                # pass B: PV chains (full 128-row mode), two j-steps per
                # head before switching banks
                ps_oe = psO.tile([P, TCH], f32, tag="ps_o", name=f"poe{ic}_{hp}")
                ps_oo = psO.tile([P, TCH], f32, tag="ps_o", name=f"poo{ic}_{hp}")
                ps_os = (ps_oe, ps_oo)
                for j0 in range(0, njt, 2):
                    with tc.high_priority(offset=500_000):
                        for h01 in range(2):
                            for jt in (j0, j0 + 1):
                                if jt >= njt:
                                    continue
                                cj, lj = jt // 4, jt % 4
                                lo = max(0, P * (jt - 4 * ic))
                                nc.tensor.matmul(
                                    ps_os[h01][0 : D + 1, lo:],
                                    vx_t[cj][:, lj, 2 * hp + h01, :],
                                    ees[jt][:, h01, lo:],
                                    start=(jt == 0),
                                    stop=(jt == njt - 1),
                                )
                    pump2(2)

                # evacuate the PV accumulators so the PSUM banks recycle
                # without waiting on the normalization chain
                oraw = []
                for h01 in range(2):
                    ow = npool.tile(
                        [D + 1, TCH], f32, tag=f"oraw{h01}", name=f"or{ic}_{hp}_{h01}"
                    )
                    nc.vector.tensor_copy(ow, ps_os[h01][0 : D + 1, :])
                    oraw.append(ow)
                return oraw

            def normalize(ic, hp, oraw):
                # rows 0..63 are O^T, row 64 the softmax sums.
                # partition_broadcast only reads physical partition 0
                # (base-64 APs return garbage on HW): DMA-hop the row.
                for h01 in range(2):
                    ow = oraw[h01]
                    stmp = npool.tile(
                        [1, TCH], f32, tag="stmp", name=f"st{ic}_{hp}_{h01}"
                    )
                    nc.sync.dma_start(stmp, ow[D : D + 1, :])
                    rb = npool.tile(
                        [D, TCH], f32, tag="rb", name=f"rb{ic}_{hp}_{h01}"
                    )
                    nc.gpsimd.partition_broadcast(rb, stmp)
                    nc.vector.reciprocal_approx_fast(rb, rb)
                    if h01 == 0:
                        nc.vector.tensor_mul(
                            ot_t[ic][0:D, hp, :], ow[0:D, :], rb
                        )
                    else:
                        tmpn = npool.tile(
                            [D, TCH], bdt, tag="tmpn", name=f"tn{ic}_{hp}"
                        )
                        nc.vector.tensor_mul(tmpn, ow[0:D, :], rb)
                        nc.sync.dma_start(ot_t[ic][D:P, hp, :], tmpn)

            def outproj_final_half(ic, half, obA):
                """Final chunk's outproj split by head-pair halves so the
                first half runs during the last head-pairs' attention."""
                otn = ot_t[ic] if half == 0 else ot_t.pop(ic)
                for tt in range(TCH // P):
                    trow = TCH * ic + P * tt
                    for mi in range(C // TCH):
                        i = 2 * tt + mi
                        msl = slice(TCH * mi, TCH * (mi + 1))
                        pso3 = ps3p.tile(
                            [P, TCH], f32, tag="pso3",
                            name=f"psf{half}_{tt}_{mi}",
                        )
                        for hp in (2 * half, 2 * half + 1):
                            nc.tensor.matmul(
                                pso3,
                                otn[:, hp, P * tt : P * (tt + 1)],
                                wots[:, hp, msl],
                                start=(hp == 2 * half),
                                stop=(hp == 2 * half + 1),
                            )
                        if half == 0:
                            obA.append(
                                ep.tile(
                                    [P, TCH], f32, tag="obA", name=f"obA{i}"
                                )
                            )
                            nc.vector.tensor_copy(obA[i], pso3)
                        else:
                            osb = ep.tile(
                                [P, TCH], bdt, tag="osb", name=f"obf{i}"
                            )
                            nc.vector.tensor_add(osb, pso3, obA[i])
                            nc.sync.dma_start(
                                out_ap[trow : trow + P, msl], osb
                            )

            # main loop: proj(0) upfront; then per chunk, attention with
            # proj(c+1)/outproj(c-1) pumped into the emission stream.
            norm_q = []
            obA = []
            for sl in range(NHP):
                for _ in proj_slice_gen(0, sl):
                    pass
            for c in range(NTCH):
                for hp in range(NHP):
                    if c + 1 < NTCH:
                        pending.append((c + 1, proj_slice_gen(c + 1, hp)))
                    norm_q.append((c, hp, attention(c, hp)))
                    normalize(*norm_q.pop(0))
                    if hp == 1 and c >= 1:
                        # ot(c-1) fully normalized at this point
                        pending.append((c, outproj_gen(c - 1)))
                    if hp == 1 and c == NTCH - 1:
                        outproj_final_half(c, 0, obA)
                if c + 1 < NTCH:
                    # attention(c+1) reads qT/kT/vx(c+1): emit the rest now
                    drain(c + 1)
            while norm_q:
                normalize(*norm_q.pop(0))
            while pending:
                pump2(100)
            outproj_final_half(NTCH - 1, 1, obA)

    nc.compile()
    return nc


def get_module(mm_fast=True):
    key = bool(mm_fast)
    if key not in _MODULE_CACHE:
        _MODULE_CACHE[key] = _build_module(key)
    return _MODULE_CACHE[key]


def make_in_maps(x, Wq, bq, Wk, bk, Wv, bv, Wo, bo):
    import ml_dtypes

    bf16 = ml_dtypes.bfloat16
    x = np.asarray(x, dtype=np.float32)
    Wq = np.asarray(Wq, dtype=np.float32)
    Wk = np.asarray(Wk, dtype=np.float32)
    Wv = np.asarray(Wv, dtype=np.float32)
    Wo = np.asarray(Wo, dtype=np.float32)
    bq = np.asarray(bq, dtype=np.float32)
    bk = np.asarray(bk, dtype=np.float32)
    bv = np.asarray(bv, dtype=np.float32)

    in_maps = []
    for core in range(NCORES):
        b, g = core // GROUPS, core % GROUPS
        gs = slice(CG * g, CG * (g + 1))
        in_maps.append(
            {
                "xt": np.ascontiguousarray(x[b].T).astype(bf16),
                "wqt": np.ascontiguousarray(Wq[gs, :].T).astype(bf16),
                "wkt": np.ascontiguousarray(Wk[gs, :].T).astype(bf16),
                "wvt": np.ascontiguousarray(Wv[gs, :].T).astype(bf16),
                "wot": np.ascontiguousarray(Wo[:, gs].T).astype(bf16),
                "bq2": np.ascontiguousarray(bq[gs].reshape(NHP, P).T),
                "bk2": np.ascontiguousarray(bk[gs].reshape(NHP, P).T),
                "bvb": np.ascontiguousarray(
                    np.broadcast_to(bv[gs][None, :], (P, CG))
                ),
            }
        )
    return in_maps


def combine_results(results, bo):
    bo = np.asarray(bo, dtype=np.float32)
    out = np.empty((B, T, C), dtype=np.float32)
    for b in range(B):
        out[b] = (
            results[GROUPS * b]["out"].astype(np.float32)
            + results[GROUPS * b + 1]["out"].astype(np.float32)
            + bo[None, :]
        )
    return out


def kernel(**inputs):
    from concourse.bass_utils import run_bass_kernel_spmd

    nc = get_module(mm_fast=True)
    in_maps = make_in_maps(
        inputs["x"],
        inputs["Wq"],
        inputs["bq"],
        inputs["Wk"],
        inputs["bk"],
        inputs["Wv"],
        inputs["bv"],
        inputs["Wo"],
        inputs["bo"],
    )
    res = run_bass_kernel_spmd(nc, in_maps, core_ids=list(range(NCORES)))
    return combine_results(res.results, inputs["bo"])
